# revision 22
# baseline (speedup 1.0000x reference)
# Trainium2 Bass kernel for nn_BDH_66056597013022 (dense_transformer).
#
# Model (per reference):
#   v = LN(emb_w[tokens])                                  [B,T,D]
#   6x: x  = relu(v @ Dx_h)            per head            [B,H,T,Dh]
#       xr = RoPE(x)
#       a  = (xr @ xr^T) @ v  ==  xr @ (xr^T @ v)          [B,H,T,D]
#       y  = relu(a @ Dy_h) * x                            [B,H,T,Dh]
#       v  = LN(v + LN(concat_h(y) @ E))
#   out = v @ readout                                      [B,T,V]
#
# Shapes: B=4 T=1024 H=4 N=4096 D=256 L=6 V=256, Dh=N/H=1024.
#
# Sharding (8 cores): core c -> batch b=c//2, head-pair hp=c%2 (heads 2hp,2hp+1).
# Cross-core coupling is only the head-sum z = y @ E, reduced over the core
# pair {2b,2b+1}; both cores then redundantly compute the LN/v-update. Even
# cores' outputs are returned.
#
# Attention is computed reassociated: Wv = xr^T @ v  [Dh,D], a = xr @ Wv.
# Since D=256 << T=1024 this is 2.5x fewer FLOPs than scores@v. It needs
# xr in both layouts ([Dh,T] from RoPE and [T,Dh] for the Wv contraction);
# the second comes from 64 PE transposes per head, interleaved with the Wv
# matmuls (k-outer) so transposes/evictions/matmuls pipeline.
#
# Schedule: everything is software-pipelined over token halves (n=0: t<512,
# n=1: t>=512) to hide the collective and the LN chain:
#   E-phase runs n-outer over both heads; the z AllReduce is split into two
#   half-T collectives, so AR(n0) overlaps E(n1) and AR(n1) overlaps the
#   n0 LN chain. The next layer's x-matmuls for half n are emitted right
#   after the half-n LN/vT rebuild, so they overlap the other half's LN.
#   RoPE for the next layer is emitted after both LN halves (the DVE order
#   is the priority order: LN chain first, RoPE last; BC-phase transposes
#   consume RoPE output ~15us later).
#
# Engine balance (DVE was the 554us co-bottleneck of the unpipelined
# version): relu evictions and the LN applies that tolerate a cross-engine
# hop go to ACT; RoPE and the PSUM evictions stay on DVE. GpSimd is NOT
# used for elementwise work: it shares SBUF ports with the DVE, and
# offloading to it measurably slowed every DVE op by ~20% while its own
# serial queue delayed v_bf (tried, reverted). PSUM bf16 evictions use
# uint32-bitcast copies (halves DVE element count). The second LN per
# layer is scale-only: its input is a sum of two LN outputs (mean 0).
#
# Everything on the matmul path is bf16 (inputs pre-cast on host, DMA'd
# directly); the residual stream v is kept f32 for the LN chain with a
# bf16 shadow copy (v_bf/vT) for matmul operands. PSUM stays fp32.
#
# On-chip layouts per core (SBUF), partition dim first:
#   v_sb  [T,D]  8x[128,256] f32      v_bf same bf16      vT [D,T] 2x[128,1024] bf16
#   xT,xr: 8x[128,1024] bf16 per head (both heads live)
#   xrs (xr std layout) 8x[128,1024] bf16;  Wv packed 2x[128,1024] bf16
#   aT [D,T] 2x[128,1024] bf16 per head (both heads live);  z [D,T] 2x[128,1024] bf16
# PSUM (8 banks): psX 4x1 rotating x/tp/aT/y/warm + boundary, psW 1x2 Wv,
# psZ 2x1 z quarters.

import os
import numpy as np

B, T, H, N, D, L, V = 4, 1024, 4, 4096, 256, 6, 256
Dh = N // H
EPS = 1e-5
NCORES = 8
P = 128
NT = T // P   # 8 token tiles
ND = D // P   # 2 model-dim tiles
NDh = Dh // P  # 8 head-dim tiles
TH = T // 2   # 512, token half

_CACHE = {}
LAST_RESULT = None


def _build_program():
    from contextlib import ExitStack

    import concourse.bass as bass
    import concourse.bacc as bacc
    import concourse.tile as tile
    import concourse.mybir as mybir
    from concourse.masks import make_identity

    f32 = mybir.dt.float32
    bf16 = mybir.dt.bfloat16
    u32 = mybir.dt.uint32
    AF = mybir.ActivationFunctionType
    ALU = mybir.AluOpType
    ts = bass.ts

    DEBUG = bool(int(os.environ.get("KERNEL_DEBUG", "0")))
    nc = bacc.Bacc("TRN2", target_bir_lowering=False, debug=False,
                   enable_asserts=False, num_devices=NCORES)

    d_oh = nc.dram_tensor("onehotT", [V, T], bf16, kind="ExternalInput").ap()
    d_ew = nc.dram_tensor("emb_w", [V, D], bf16, kind="ExternalInput").ap()
    d_cos = nc.dram_tensor("cosT", [Dh // 2, T], bf16, kind="ExternalInput").ap()
    d_sin = nc.dram_tensor("sinT", [Dh // 2, T], bf16, kind="ExternalInput").ap()
    d_dx = nc.dram_tensor("dx", [2 * D, Dh], bf16, kind="ExternalInput").ap()
    d_dy = nc.dram_tensor("dy", [2 * D, Dh], bf16, kind="ExternalInput").ap()
    d_eh = nc.dram_tensor("eh", [2 * Dh, D], bf16, kind="ExternalInput").ap()
    d_ro = nc.dram_tensor("readout", [D, V], bf16, kind="ExternalInput").ap()
    d_out = nc.dram_tensor("out", [T, V], f32, kind="ExternalOutput").ap()
    d_dbg = {}
    if DEBUG:
        for nm, shp in [("v0", [T, D]), ("xr00", [P, T]), ("xrs0", [P, Dh]),
                        ("wv0", [P, T]), ("aT00", [P, T]), ("y00", [P, 512]),
                        ("z0", [P, T]), ("zq0", [P, T]), ("v1", [T, D])]:
            d_dbg[nm] = nc.dram_tensor(
                f"dbg_{nm}", shp, f32, kind="ExternalOutput").ap()

    with tile.TileContext(nc) as tc, ExitStack() as ctx:
        wpool = ctx.enter_context(tc.tile_pool(name="weights", bufs=1))
        vpool = ctx.enter_context(tc.tile_pool(name="vpool", bufs=1))
        # xT/xr: 8 tiles per head, both heads live across the whole layer
        xpool = ctx.enter_context(tc.tile_pool(name="xpool", bufs=16))
        xrpool = ctx.enter_context(tc.tile_pool(name="xrpool", bufs=16))
        xspool = ctx.enter_context(tc.tile_pool(name="xspool", bufs=8))
        wvpool = ctx.enter_context(tc.tile_pool(name="wvpool", bufs=2))
        apool = ctx.enter_context(tc.tile_pool(name="apool", bufs=4))
        yrpool = ctx.enter_context(tc.tile_pool(name="yrpool", bufs=3))
        ypool = ctx.enter_context(tc.tile_pool(name="ypool", bufs=4))
        zpool = ctx.enter_context(tc.tile_pool(name="zpool", bufs=1))
        zrpool = ctx.enter_context(tc.tile_pool(name="zrpool", bufs=4))
        zqpool = ctx.enter_context(tc.tile_pool(name="zqpool", bufs=2))
        lnpool = ctx.enter_context(tc.tile_pool(name="lnpool", bufs=3))
        stpool = ctx.enter_context(tc.tile_pool(name="stpool", bufs=4))
        # PSUM budget (8 banks): psX 4x1 + psW 1x2 + psZ 2x1 = 8
        psX = ctx.enter_context(tc.tile_pool(name="psX", bufs=4, space="PSUM"))
        psW = ctx.enter_context(tc.tile_pool(name="psW", bufs=1, space="PSUM"))
        psZ = ctx.enter_context(tc.tile_pool(name="psZ", bufs=2, space="PSUM"))
        dpool = ctx.enter_context(tc.tile_pool(name="drampool", bufs=2, space="DRAM"))

        # ---- persistent weights: direct bf16 DMA, no round-copies ----
        oh_sb = []
        for k in range(ND):
            t_ = wpool.tile([P, T], bf16, tag=f"oh{k}", name=f"oh{k}")
            nc.sync.dma_start(t_[:], d_oh[ts(k, P), :])
            oh_sb.append(t_)
        ew_sb = []
        for k in range(ND):
            t_ = wpool.tile([P, D], bf16, tag=f"ew{k}", name=f"ew{k}")
            nc.sync.dma_start(t_[:], d_ew[ts(k, P), :])
            ew_sb.append(t_)
        cos_sb = []
        sin_sb = []
        for i in range(4):
            t_ = wpool.tile([P, T], bf16, tag=f"cos{i}", name=f"cos{i}")
            nc.sync.dma_start(t_[:], d_cos[ts(i, P), :])
            cos_sb.append(t_)
        for i in range(4):
            t_ = wpool.tile([P, T], bf16, tag=f"sin{i}", name=f"sin{i}")
            nc.sync.dma_start(t_[:], d_sin[ts(i, P), :])
            sin_sb.append(t_)
        dx_sb = []
        for i in range(4):
            t_ = wpool.tile([P, Dh], bf16, tag=f"dx{i}", name=f"dx{i}")
            nc.sync.dma_start(t_[:], d_dx[ts(i, P), :])
            dx_sb.append(t_)
        dy_sb = []
        for i in range(4):
            t_ = wpool.tile([P, Dh], bf16, tag=f"dy{i}", name=f"dy{i}")
            nc.sync.dma_start(t_[:], d_dy[ts(i, P), :])
            dy_sb.append(t_)
        eh_sb = []  # eh_sb[j][k]: E rows for local head j, e-block k
        for j in range(2):
            row = []
            for k in range(NDh):
                t_ = wpool.tile([P, D], bf16, tag=f"eh{j}_{k}", name=f"eh{j}_{k}")
                nc.sync.dma_start(t_[:], d_eh[ts(8 * j + k, P), :])
                row.append(t_)
            eh_sb.append(row)
        ro_sb = []
        for k in range(ND):
            t_ = wpool.tile([P, V], bf16, tag=f"ro{k}", name=f"ro{k}")
            nc.sync.dma_start(t_[:], d_ro[ts(k, P), :])
            ro_sb.append(t_)

        ident = wpool.tile([P, P], f32, tag="ident", name="ident")
        make_identity(nc, ident)
        identb = wpool.tile([P, P], bf16, tag="identb", name="identb")
        nc.scalar.copy(identb[:], ident[:])
        epsc = wpool.tile([P, 1], f32, tag="epsc", name="epsc")
        nc.gpsimd.memset(epsc[:], EPS)
        warmsink = wpool.tile([P, 1], f32, tag="warmsink", name="warmsink")

        def warm(n_mms, label, width=512):
            # HAM re-throttles the PE to 1.2 GHz after ~3.4us idle; feed it
            # dependency-free matmuls during known stall windows.
            wps = psX.tile([P, width], f32, tag="psX", name=f"warm_{label}")
            for i in range(n_mms):
                nc.tensor.matmul(wps[:], dx_sb[0][:, 0:P], dx_sb[1][:, 0:width],
                                 start=(i == 0), stop=(i == n_mms - 1))
            nc.scalar.copy(warmsink[:], wps[:, 0:1])

        def warm_on(dep_ap, label, n_mms=2, width=256):
            # warm matmuls that WAIT on dep_ap: naturally spaced through a
            # stall window by the producer chain of dep_ap.
            wps = psX.tile([P, width], f32, tag="psX", name=f"warmd_{label}")
            for i in range(n_mms):
                nc.tensor.matmul(wps[:], dx_sb[0][:, 0:P], dep_ap,
                                 start=(i == 0), stop=(i == n_mms - 1))
            nc.scalar.copy(warmsink[:], wps[:, 0:1])

        # ---- persistent activations ----
        v_sb = [vpool.tile([P, D], f32, tag=f"v{m}", name=f"v{m}")
                for m in range(NT)]
        v_bf = [vpool.tile([P, D], bf16, tag=f"vb{m}", name=f"vb{m}")
                for m in range(NT)]
        vT_sb = [vpool.tile([P, T], bf16, tag=f"vT{k}", name=f"vT{k}")
                 for k in range(ND)]

        def ln_stats(src_ap):
            st6 = stpool.tile([P, 6], f32, tag="st6", name="st6")
            nc.vector.bn_stats(st6[:], src_ap)
            mv = stpool.tile([P, 2], f32, tag="mv", name="mv")
            nc.vector.bn_aggr(mv[:], st6[:])
            sd = stpool.tile([P, 1], f32, tag="sd", name="sd")
            nc.scalar.activation(sd[:], mv[:, 1:2], AF.Sqrt, bias=epsc[:], scale=1.0)
            rstd = stpool.tile([P, 1], f32, tag="rstd", name="rstd")
            nc.vector.reciprocal(rstd[:], sd[:])
            nmr = stpool.tile([P, 1], f32, tag="nmr", name="nmr")
            nc.vector.scalar_tensor_tensor(
                nmr[:], mv[:, 0:1], -1.0, rstd[:], op0=ALU.mult, op1=ALU.mult)
            return rstd, nmr

        def layer_norm_dual(src_ap, m, zero_mean=False):
            # v_sb[m] (f32, on ACT) and v_bf[m] (bf16, on DVE) from one stats
            # pass. Both applies are off the serial LN chain (consumers are
            # next layer's matmuls). zero_mean: src is a sum of two LN
            # outputs, so its mean is 0 and the applies are scale-only.
            rstd, nmr = ln_stats(src_ap)
            if zero_mean:
                nc.scalar.activation(v_sb[m][:], src_ap, AF.Identity,
                                     bias=0.0, scale=rstd[:])
                nc.vector.tensor_scalar(v_bf[m][:], src_ap, rstd[:], None,
                                        op0=ALU.mult)
            else:
                nc.scalar.activation(v_sb[m][:], src_ap, AF.Identity,
                                     bias=nmr[:], scale=rstd[:])
                nc.vector.tensor_scalar(v_bf[m][:], src_ap, rstd[:], nmr[:],
                                        op0=ALU.mult, op1=ALU.add)

        def dump(dst_ap, src_ap, name="dbg"):
            stg = lnpool.tile([P, src_ap.free_size()], f32, tag="dbgstg",
                              name=f"stg_{name}")
            nc.scalar.copy(stg[:], src_ap)
            nc.sync.dma_start(dst_ap, stg[:])

        def transpose_v_quarter(q):
            # vT[:, q*256:(q+1)*256] <- v_bf[2q], v_bf[2q+1]
            for kd in range(ND):
                tps = psX.tile([P, D], bf16, tag="psX", name=f"vtp{kd}")
                for mi in range(2):
                    m = 2 * q + mi
                    nc.tensor.transpose(
                        tps[:, ts(mi, P)], v_bf[m][:, ts(kd, P)], identb[:])
                nc.vector.tensor_copy(
                    vT_sb[kd][:, ts(q, D)].bitcast(u32), tps[:].bitcast(u32))

        def transpose_v_half(n):
            transpose_v_quarter(2 * n)
            transpose_v_quarter(2 * n + 1)

        # xT[j][mm] / xr[j][mm] for the CURRENT layer (rotated via pools)
        xT = [[None] * NDh, [None] * NDh]
        xr = [[None] * NDh, [None] * NDh]

        def emit_A_quarter(j, q, layer):
            # x[:, q*256:(q+1)*256] = relu(Dx^T @ vT[:, quarter]) for head j
            for mp in range(4):
                for mm in (mp, mp + 4):
                    xps = psX.tile([P, D], f32, tag="psX", name="xps")
                    for k in range(ND):
                        nc.tensor.matmul(
                            xps[:], dx_sb[2 * j + k][:, ts(mm, P)],
                            vT_sb[k][:, ts(q, D)],
                            start=(k == 0), stop=(k == ND - 1))
                    if q == 0:
                        xT[j][mm] = xpool.tile([P, T], bf16, tag="xT",
                                               name=f"xT{j}_{mm}_{layer}")
                    nc.scalar.activation(xT[j][mm][:, ts(q, D)], xps[:], AF.Relu)

        def emit_A_half(j, n, layer):
            emit_A_quarter(j, 2 * n, layer)
            emit_A_quarter(j, 2 * n + 1, layer)

        def emit_rope_half(j, mp, n, layer):
            # RoPE on the (mp, mp+4) tile pair, token half n, all bf16 DVE:
            #   xr_lo = lo*cos - hi*sin ;  xr_hi = hi*cos + lo*sin
            sl = bass.ts(n, TH)
            cm, sm = cos_sb[mp][:, sl], sin_sb[mp][:, sl]
            lo, hi = xT[j][mp], xT[j][mp + 4]
            if n == 0:
                xr[j][mp] = xrpool.tile([P, T], bf16, tag="xr",
                                        name=f"xr{j}_{mp}_{layer}")
                xr[j][mp + 4] = xrpool.tile([P, T], bf16, tag="xr",
                                            name=f"xr{j}_{mp + 4}_{layer}")
            xrl, xrh = xr[j][mp], xr[j][mp + 4]
            t1 = stpool.tile([P, TH], bf16, tag="ropetmp", name="rt1")
            nc.vector.tensor_mul(t1[:], hi[:, sl], sm)
            nc.vector.tensor_mul(xrl[:, sl], lo[:, sl], cm)
            nc.vector.tensor_sub(xrl[:, sl], xrl[:, sl], t1[:])
            t2 = stpool.tile([P, TH], bf16, tag="ropetmp", name="rt2")
            nc.vector.tensor_mul(t2[:], lo[:, sl], sm)
            nc.vector.tensor_mul(xrh[:, sl], hi[:, sl], cm)
            nc.vector.tensor_add(xrh[:, sl], xrh[:, sl], t2[:])

        def emit_ropes_head(j, layer):
            for n in range(2):
                for mp in range(4):
                    emit_rope_half(j, mp, n, layer)

        # ---- embedding: v0 = LN(onehot @ emb_w) ----
        for m in range(NT):
            eps_t = psX.tile([P, D], f32, tag="psX", name="embps")
            for k in range(ND):
                nc.tensor.matmul(eps_t[:], oh_sb[k][:, ts(m, P)], ew_sb[k][:],
                                 start=(k == 0), stop=(k == ND - 1))
            emb_t = lnpool.tile([P, D], f32, tag="w", name="embt")
            nc.scalar.copy(emb_t[:], eps_t[:])
            layer_norm_dual(emb_t[:], m)
            if DEBUG:
                dump(d_dbg["v0"][ts(m, P), :], v_sb[m][:], name=f"v0_{m}")
        transpose_v_half(0)
        transpose_v_half(1)
        for nn_ in range(2):
            emit_A_half(0, nn_, 0)
            emit_A_half(1, nn_, 0)
        emit_ropes_head(0, 0)

        rg = [[0, 1], [2, 3], [4, 5], [6, 7]]

        # Prime the collective rings during the lead-in: the first real
        # AllReduce otherwise pays a one-time ~13us setup stall.
        pin = dpool.tile([P, P], bf16, tag="prime_i", name="prime_i")
        pout = dpool.tile([P, P], bf16, tag="prime_o", name="prime_o")
        nc.sync.dma_start(pin[:], identb[:])
        nc.gpsimd.collective_compute(
            "AllReduce", mybir.AluOpType.add,
            ins=[pin.opt()], outs=[pout.opt()], replica_groups=rg)

        for layer in range(L):
            z_sb = [zpool.tile([P, T], bf16, tag=f"z{i}", name=f"z{i}_{layer}")
                    for i in range(2)]
            zin = [dpool.tile([2 * P, TH], bf16, tag=f"zin{n}",
                              name=f"zin{n}_{layer}") for n in range(2)]
            zout = [dpool.tile([2 * P, TH], bf16, tag=f"zout{n}",
                               name=f"zout{n}_{layer}") for n in range(2)]
            if DEBUG and layer == 0:
                dump(d_dbg["xr00"][:], xr[0][0][:], name="xr00")

            aT = [[None, None], [None, None]]
            for j in range(2):
                if j == 1:
                    # head1 RoPE just-in-time: its DVE ops queue behind
                    # head0's xrs evictions, ready when BC(j1) needs them
                    emit_ropes_head(1, layer)
                # ---- B/C: xr_std = tp(xr) interleaved k-outer with
                #           Wv = xr^T @ v  (psW pass1 e-blocks 0-3, pass2 4-7)
                xrs = [None] * NT
                wv_sb = [None, None]
                for half in range(2):
                    wvp = psW.tile([P, T], f32, tag="psW", name=f"wv{half}")
                    wvt = wvpool.tile([P, T], bf16, tag="wv", name=f"wv{half}")
                    # eb-outer: each e-block's 8-mm accumulation chain is
                    # contiguous. Interleaving chains that share a PSUM bank
                    # is WRONG: start=True clears has_written for the WHOLE
                    # bank, so a sibling chain's k=0 contribution gets
                    # overwritten at its k=1. Transposes (non-accumulating,
                    # other banks) interleave freely with the first chain.
                    # The tp's for Dh-half `half` are emitted inside pass
                    # `half`, so pass 0 starts as soon as the lo xr tiles
                    # are RoPE'd. Bank 0 (eb 0-1) is evicted while eb 2-3
                    # still accumulate in bank 1 (different banks: legal).
                    for eb in range(4):
                        e = 4 * half + eb
                        for k in range(NT):
                            if eb == 0:
                                tpp = psX.tile([P, TH], bf16, tag="psX",
                                               name=f"tpp{k}")
                                for kk in range(4):
                                    nc.tensor.transpose(
                                        tpp[:, ts(kk, P)],
                                        xr[j][4 * half + kk][:, ts(k, P)],
                                        identb[:])
                                if half == 0:
                                    xrs[k] = xspool.tile(
                                        [P, Dh], bf16, tag="xrs", name=f"xrs{k}")
                                nc.vector.tensor_copy(
                                    xrs[k][:, ts(half, TH)].bitcast(u32),
                                    tpp[:].bitcast(u32))
                                if DEBUG and layer == 0 and j == 0 and k == 0 \
                                        and half == 1:
                                    dump(d_dbg["xrs0"][:], xrs[k][:],
                                         name="xrs0")
                            nc.tensor.matmul(
                                wvp[:, ts(eb, D)],
                                xrs[k][:, ts(e, P)],
                                v_bf[k][:],
                                start=(k == 0), stop=(k == NT - 1))
                        if eb == 1:
                            nc.scalar.copy(wvt[:, 0:TH], wvp[:, 0:TH])
                    nc.scalar.copy(wvt[:, TH:T], wvp[:, TH:T])
                    wv_sb[half] = wvt
                    if DEBUG and layer == 0 and j == 0 and half == 0:
                        dump(d_dbg["wv0"][:], wvt[:], name="wv0")

                # ---- D: aT = (xr @ Wv)^T = Wv^T-as-lhsT @ xr, per n-half ----
                for m in range(ND):
                    aT[j][m] = apool.tile([P, T], bf16, tag="aT",
                                          name=f"aT{j}_{m}_{layer}")
                for n in range(2):
                    for m in range(ND):
                        atp = psX.tile([P, TH], f32, tag="psX", name=f"atp{m}")
                        for k in range(NDh):
                            nc.tensor.matmul(
                                atp[:],
                                wv_sb[k // 4][:, (k % 4) * D + m * P:
                                              (k % 4) * D + (m + 1) * P],
                                xr[j][k][:, ts(n, TH)],
                                start=(k == 0), stop=(k == NDh - 1))
                        nc.scalar.copy(aT[j][m][:, ts(n, TH)], atp[:])
                if DEBUG and layer == 0 and j == 0:
                    dump(d_dbg["aT00"][:], aT[j][0][:], name="aT00")

            # ---- E: y = relu(Dy^T @ aT) * x ; z += E_h^T @ y. n-outer over
            #      both heads so z[:, n0] completes early and its AllReduce
            #      overlaps the n1 compute. ----
            for n in range(2):
                for j in range(2):
                    z_ps = [psZ.tile([P, TH], f32, tag="psZ", name=f"zps{i}")
                            for i in range(2)]
                    y_half = [None] * NDh

                    def emit_zn(k, j=j, z_ps=z_ps, y_half=y_half):
                        for m in range(ND):
                            nc.tensor.matmul(
                                z_ps[m][:],
                                eh_sb[j][k][:, ts(m, P)],
                                y_half[k][:],
                                start=(k == 0), stop=(k == NDh - 1))

                    for k in range(NDh):
                        yps = psX.tile([P, TH], f32, tag="psX", name="yps")
                        for kk in range(ND):
                            nc.tensor.matmul(
                                yps[:],
                                dy_sb[2 * j + kk][:, ts(k, P)],
                                aT[j][kk][:, ts(n, TH)],
                                start=(kk == 0), stop=(kk == ND - 1))
                        yr = yrpool.tile([P, TH], bf16, tag="yr", name=f"yr{k}")
                        nc.scalar.activation(yr[:], yps[:], AF.Relu)
                        yh = ypool.tile([P, TH], bf16, tag="y", name=f"y{k}")
                        nc.vector.tensor_mul(yh[:], yr[:], xT[j][k][:, ts(n, TH)])
                        y_half[k] = yh
                        if DEBUG and layer == 0 and j == 0 and n == 0 and k == 0:
                            dump(d_dbg["y00"][:], yh[:], name="y00")
                        if k > 0:
                            emit_zn(k - 1)
                    emit_zn(NDh - 1)
                    if j == 0:
                        for i in range(2):
                            nc.scalar.copy(z_sb[i][:, ts(n, TH)], z_ps[i][:])
                    else:
                        # the local j0+j1 sum and its DMA gate the AllReduce
                        # launch: keep them ahead of the fusion backlog
                        with tc.high_priority():
                            for i in range(2):
                                nc.vector.scalar_tensor_tensor(
                                    z_sb[i][:, ts(n, TH)], z_ps[i][:], 0.0,
                                    z_sb[i][:, ts(n, TH)],
                                    op0=ALU.add, op1=ALU.add)
                                nc.sync.dma_start(zin[n][ts(i, P), :],
                                                  z_sb[i][:, ts(n, TH)])
                # half-T AllReduce of zT over the core pair, fired as soon
                # as this half's z is complete
                nc.gpsimd.collective_compute(
                    "AllReduce", mybir.AluOpType.add,
                    ins=[zin[n].opt()], outs=[zout[n].opt()],
                    replica_groups=rg)

            if DEBUG and layer == 0:
                dump(d_dbg["z0"][:], z_sb[0][:], name="z0")

            warm(6, f"ar{layer}")

            # ---- boundary: zq transpose + LN chain per token half, with
            #      half n1's LN running on DVE while the PE does half n0's
            #      vT rebuild + next-layer x-phase. PE order: zq0-tp,
            #      (warm-paced LN m0-1), zq1-tp, (LN m2-3), vT0, A'(n0),
            #      vT1, A'(n1). DVE order: zq0-ev, LN m0-3, zq1-ev, vT0-ev,
            #      LN m4-7, vT1-ev, ropes. ----
            last = (layer == L - 1)
            zr = [[zrpool.tile([P, TH], bf16, tag=f"zr{i}",
                               name=f"zr{i}_{n}_{layer}") for i in range(2)]
                  for n in range(2)]
            for n in range(2):
                for i in range(2):
                    nc.sync.dma_start(zr[n][i][:], zout[n][ts(i, P), :])
            zqt = [None, None]
            zS = {}

            def zq_transpose(n):
                zqp = psX.tile([P, T], bf16, tag="psX", name=f"zqp{n}")
                for mloc in range(4):
                    for kd in range(ND):
                        nc.tensor.transpose(
                            zqp[:, mloc * D + kd * P: mloc * D + (kd + 1) * P],
                            zr[n][kd][:, ts(mloc, P)], identb[:])
                zqt[n] = zqpool.tile([P, T], bf16, tag="zq",
                                     name=f"zq{n}_{layer}")
                # even m-blocks (DVE LN chains) evicted on DVE; odd m-blocks
                # (ACT LN chains) on ACT with the row-sum accumulated for
                # their mean as a side effect
                for mloc in (0, 2):
                    nc.vector.tensor_copy(
                        zqt[n][:, ts(mloc, D)].bitcast(u32),
                        zqp[:, ts(mloc, D)].bitcast(u32))
                for mloc in (1, 3):
                    m = 4 * n + mloc
                    S = stpool.tile([P, 1], f32, tag="S", name=f"S{m}")
                    nc.scalar.activation(
                        zqt[n][:, ts(mloc, D)], zqp[:, ts(mloc, D)],
                        AF.Identity, bias=0.0, scale=1.0, accum_out=S[:])
                    zS[m] = S

            def ln_m(m):
                # DVE-resident LN chain (bn_stats path)
                zb = zqt[m // 4][:, ts(m % 4, D)]
                rstd, nmr = ln_stats(zb)
                u = lnpool.tile([P, D], f32, tag="u", name=f"u{m}")
                nc.vector.tensor_scalar(u[:], zb, rstd[:], nmr[:],
                                        op0=ALU.mult, op1=ALU.add)
                w = lnpool.tile([P, D], f32, tag="w", name=f"w{m}")
                nc.vector.tensor_add(w[:], v_sb[m][:], u[:])
                # w = LN(z) + v where v is itself an LN output, so
                # mean(w) = 0 and the second LN is scale-only
                layer_norm_dual(w[:], m, zero_mean=True)
                warm_on(v_bf[m][:], f"ln{layer}_{m}")
                if DEBUG and layer == 0:
                    dump(d_dbg["v1"][ts(m, P), :], v_sb[m][:], name=f"v1_{m}")

            def ln_m_act(m):
                # ACT-resident LN chain, running in parallel with the DVE
                # chains: stats via accum_out row-sums + Square, with mean^2
                # folded into the Sqrt bias. sqrt(SSQ/D + (eps - mean^2))
                # == sqrt(var + eps).
                n, mloc = m // 4, m % 4
                zb = zqt[n][:, ts(mloc, D)]
                S = zS[m]
                sq = lnpool.tile([P, D], f32, tag="sqscr", name=f"sq{m}")
                ssq = stpool.tile([P, 1], f32, tag="SSQ", name=f"SSQ{m}")
                nc.scalar.activation(sq[:], zb, AF.Square, accum_out=ssq[:])
                m2 = stpool.tile([P, 1], f32, tag="m2", name=f"m2{m}")
                nc.scalar.activation(m2[:], S[:], AF.Square, scale=1.0 / D)
                negb = stpool.tile([P, 1], f32, tag="negb", name=f"negb{m}")
                nc.scalar.activation(negb[:], m2[:], AF.Identity,
                                     bias=epsc[:], scale=-1.0)
                sd = stpool.tile([P, 1], f32, tag="sd", name=f"sd{m}")
                nc.scalar.activation(sd[:], ssq[:], AF.Sqrt,
                                     bias=negb[:], scale=1.0 / D)
                rstd = stpool.tile([P, 1], f32, tag="rstd", name=f"rstd{m}")
                nc.vector.reciprocal(rstd[:], sd[:])
                nmr = stpool.tile([P, 1], f32, tag="nmr", name=f"nmr{m}")
                nc.vector.scalar_tensor_tensor(
                    nmr[:], S[:], -1.0 / D, rstd[:], op0=ALU.mult, op1=ALU.mult)
                u = lnpool.tile([P, D], f32, tag="u", name=f"u{m}")
                nc.scalar.activation(u[:], zb, AF.Identity,
                                     bias=nmr[:], scale=rstd[:])
                w = lnpool.tile([P, D], f32, tag="w", name=f"w{m}")
                nc.vector.tensor_add(w[:], v_sb[m][:], u[:])
                # second LN scale-only (mean(w) = 0), stats on ACT
                sqw = lnpool.tile([P, D], f32, tag="sqscr", name=f"sqw{m}")
                ssqw = stpool.tile([P, 1], f32, tag="SSQ", name=f"SSQw{m}")
                nc.scalar.activation(sqw[:], w[:], AF.Square, accum_out=ssqw[:])
                sdw = stpool.tile([P, 1], f32, tag="sd", name=f"sdw{m}")
                nc.scalar.activation(sdw[:], ssqw[:], AF.Sqrt,
                                     bias=epsc[:], scale=1.0 / D)
                rstdw = stpool.tile([P, 1], f32, tag="rstd", name=f"rstdw{m}")
                nc.vector.reciprocal(rstdw[:], sdw[:])
                nc.scalar.activation(v_sb[m][:], w[:], AF.Identity,
                                     bias=0.0, scale=rstdw[:])
                nc.vector.tensor_scalar(v_bf[m][:], w[:], rstdw[:], None,
                                        op0=ALU.mult)
                warm_on(v_bf[m][:], f"lna{layer}_{m}")
                if DEBUG and layer == 0:
                    dump(d_dbg["v1"][ts(m, P), :], v_sb[m][:], name=f"v1_{m}")

            def emit_next_quarter(q):
                if last:
                    for mi in range(2):
                        m = 2 * q + mi
                        rps = psX.tile([P, V], f32, tag="psX", name="rps")
                        for k in range(ND):
                            nc.tensor.matmul(
                                rps[:], vT_sb[k][:, ts(m, P)], ro_sb[k][:],
                                start=(k == 0), stop=(k == ND - 1))
                        o_sb = lnpool.tile([P, V], f32, tag="o", name=f"o{m}")
                        nc.scalar.copy(o_sb[:], rps[:])
                        nc.sync.dma_start(d_out[ts(m, P), :], o_sb[:])
                else:
                    emit_A_quarter(0, q, layer + 1)
                    emit_A_quarter(1, q, layer + 1)

            # The LN chain is the serial critical path of the boundary; give
            # it scheduler priority so the ready-heap doesn't interleave
            # next-layer RoPE/relu work ahead of it on DVE/ACT. Quarter
            # granularity: each pair of LN chains (one DVE, one ACT)
            # unlocks a vT quarter and its next-layer x-matmul chunk.
            for q in range(4):
                with tc.high_priority():
                    if q == 0:
                        zq_transpose(0)
                    if q == 2:
                        zq_transpose(1)
                    ln_m(2 * q)
                    ln_m_act(2 * q + 1)
                    transpose_v_quarter(q)
                    if DEBUG and layer == 0 and q == 1:
                        dump(d_dbg["zq0"][:], zqt[0][:], name="zq0")
                emit_next_quarter(q)
                if not last and q == 1:
                    for mp in range(4):
                        emit_rope_half(0, mp, 0, layer + 1)
            if not last:
                for mp in range(4):
                    emit_rope_half(0, mp, 1, layer + 1)

    nc.compile()
    return nc


def _get_program():
    if "nc" not in _CACHE:
        _CACHE["nc"] = _build_program()
    return _CACHE["nc"]


def _rope_tables():
    inv = (1.0 / (10000.0 ** (np.arange(0, Dh, 2, dtype=np.float32) / Dh)))
    tt = np.arange(T, dtype=np.float32)
    freqs = np.outer(tt, inv).astype(np.float32)  # [T, Dh/2]
    cosT = np.ascontiguousarray(np.cos(freqs).T)
    sinT = np.ascontiguousarray(np.sin(freqs).T)
    return cosT, sinT


def kernel(**inputs):
    global LAST_RESULT
    import ml_dtypes
    from concourse import bass_utils

    bf = ml_dtypes.bfloat16
    tokens = np.asarray(inputs["tokens"])
    emb_w = np.ascontiguousarray(inputs["emb_w"], dtype=np.float32)
    E = np.ascontiguousarray(inputs["E"], dtype=np.float32)
    Dx = np.ascontiguousarray(inputs["Dx"], dtype=np.float32)
    Dy = np.ascontiguousarray(inputs["Dy"], dtype=np.float32)
    readout = np.ascontiguousarray(inputs["readout"], dtype=np.float32)

    cosT, sinT = _rope_tables()

    in_maps = []
    for c in range(NCORES):
        b, hp = c // 2, c % 2
        oh = np.zeros((V, T), dtype=np.float32)
        oh[np.asarray(tokens[b], dtype=np.int64), np.arange(T)] = 1.0
        in_maps.append({
            "onehotT": oh.astype(bf),
            "emb_w": emb_w.astype(bf),
            "cosT": cosT.astype(bf),
            "sinT": sinT.astype(bf),
            "dx": np.ascontiguousarray(
                Dx[2 * hp:2 * hp + 2].reshape(2 * D, Dh)).astype(bf),
            "dy": np.ascontiguousarray(
                Dy[2 * hp:2 * hp + 2].reshape(2 * D, Dh)).astype(bf),
            "eh": np.ascontiguousarray(
                E[2 * hp * Dh:(2 * hp + 2) * Dh]).astype(bf),
            "readout": readout.astype(bf),
        })

    nc = _get_program()
    res = bass_utils.run_bass_kernel_spmd(
        nc, in_maps, core_ids=list(range(NCORES)),
        trace=bool(int(os.environ.get("KERNEL_TRACE", "0"))))
    LAST_RESULT = res
    out = np.stack([res.results[2 * b]["out"] for b in range(B)], axis=0)
    return out


# revision 25
# speedup vs baseline: 1.1050x; 1.1050x over previous
# Trainium2 Bass kernel for nn_BDH_66056597013022 (dense_transformer).
#
# Model (per reference):
#   v = LN(emb_w[tokens])                                  [B,T,D]
#   6x: x  = relu(v @ Dx_h)            per head            [B,H,T,Dh]
#       xr = RoPE(x)
#       a  = (xr @ xr^T) @ v  ==  xr @ (xr^T @ v)          [B,H,T,D]
#       y  = relu(a @ Dy_h) * x                            [B,H,T,Dh]
#       v  = LN(v + LN(concat_h(y) @ E))
#   out = v @ readout                                      [B,T,V]
#
# Shapes: B=4 T=1024 H=4 N=4096 D=256 L=6 V=256, Dh=N/H=1024.
#
# Sharding (8 cores): core c -> batch b=c//2, head-pair hp=c%2 (heads 2hp,2hp+1).
# Cross-core coupling is only the head-sum z = y @ E, reduced over the core
# pair {2b,2b+1}; both cores then redundantly compute the LN/v-update. Even
# cores' outputs are returned.
#
# Attention is computed reassociated: Wv = xr^T @ v  [Dh,D], a = xr @ Wv.
# Since D=256 << T=1024 this is 2.5x fewer FLOPs than scores@v. It needs
# xr in both layouts ([Dh,T] from RoPE and [T,Dh] for the Wv contraction);
# the second comes from 64 PE transposes per head, interleaved with the Wv
# matmuls (k-outer) so transposes/evictions/matmuls pipeline.
#
# Schedule: everything is software-pipelined over token halves (n=0: t<512,
# n=1: t>=512) to hide the collective and the LN chain:
#   E-phase runs n-outer over both heads; the z AllReduce is split into two
#   half-T collectives, so AR(n0) overlaps E(n1) and AR(n1) overlaps the
#   n0 LN chain. The next layer's x-matmuls for half n are emitted right
#   after the half-n LN/vT rebuild, so they overlap the other half's LN.
#   RoPE for the next layer is emitted after both LN halves (the DVE order
#   is the priority order: LN chain first, RoPE last; BC-phase transposes
#   consume RoPE output ~15us later).
#
# Engine balance (DVE was the 554us co-bottleneck of the unpipelined
# version): relu evictions and the LN applies that tolerate a cross-engine
# hop go to ACT; RoPE and the PSUM evictions stay on DVE. GpSimd is NOT
# used for elementwise work: it shares SBUF ports with the DVE, and
# offloading to it measurably slowed every DVE op by ~20% while its own
# serial queue delayed v_bf (tried, reverted). PSUM bf16 evictions use
# uint32-bitcast copies (halves DVE element count). The second LN per
# layer is scale-only: its input is a sum of two LN outputs (mean 0).
#
# Everything on the matmul path is bf16 (inputs pre-cast on host, DMA'd
# directly); the residual stream v is kept f32 for the LN chain with a
# bf16 shadow copy (v_bf/vT) for matmul operands. PSUM stays fp32.
#
# On-chip layouts per core (SBUF), partition dim first:
#   v_sb  [T,D]  8x[128,256] f32      v_bf same bf16      vT [D,T] 2x[128,1024] bf16
#   xT,xr: 8x[128,1024] bf16 per head (both heads live)
#   xrs (xr std layout) 8x[128,1024] bf16;  Wv packed 2x[128,1024] bf16
#   aT [D,T] 2x[128,1024] bf16 per head (both heads live);  z [D,T] 2x[128,1024] bf16
# PSUM (8 banks): psX 4x1 rotating x/tp/aT/y/warm + boundary, psW 1x2 Wv,
# psZ 2x1 z quarters.

import os
import numpy as np

B, T, H, N, D, L, V = 4, 1024, 4, 4096, 256, 6, 256
Dh = N // H
EPS = 1e-5
NCORES = 8
P = 128
NT = T // P   # 8 token tiles
ND = D // P   # 2 model-dim tiles
NDh = Dh // P  # 8 head-dim tiles
TH = T // 2   # 512, token half

_CACHE = {}
LAST_RESULT = None


def _build_program():
    from contextlib import ExitStack

    import concourse.bass as bass
    import concourse.bacc as bacc
    import concourse.tile as tile
    import concourse.mybir as mybir
    from concourse.masks import make_identity

    f32 = mybir.dt.float32
    bf16 = mybir.dt.bfloat16
    u32 = mybir.dt.uint32
    AF = mybir.ActivationFunctionType
    ALU = mybir.AluOpType
    ts = bass.ts

    DEBUG = bool(int(os.environ.get("KERNEL_DEBUG", "0")))
    nc = bacc.Bacc("TRN2", target_bir_lowering=False, debug=False,
                   enable_asserts=False, num_devices=NCORES)

    d_oh = nc.dram_tensor("onehotT", [V, T], bf16, kind="ExternalInput").ap()
    d_ew = nc.dram_tensor("emb_w", [V, D], bf16, kind="ExternalInput").ap()
    d_cos = nc.dram_tensor("cosT", [Dh // 2, T], bf16, kind="ExternalInput").ap()
    d_sin = nc.dram_tensor("sinT", [Dh // 2, T], bf16, kind="ExternalInput").ap()
    d_dx = nc.dram_tensor("dx", [2 * D, Dh], bf16, kind="ExternalInput").ap()
    d_dy = nc.dram_tensor("dy", [2 * D, Dh], bf16, kind="ExternalInput").ap()
    d_eh = nc.dram_tensor("eh", [2 * Dh, D], bf16, kind="ExternalInput").ap()
    d_ro = nc.dram_tensor("readout", [D, V], bf16, kind="ExternalInput").ap()
    d_out = nc.dram_tensor("out", [T, V], f32, kind="ExternalOutput").ap()
    d_dbg = {}
    if DEBUG:
        for nm, shp in [("v0", [T, D]), ("xr00", [P, T]), ("xrs0", [P, Dh]),
                        ("wv0", [P, T]), ("aT00", [P, T]), ("y00", [P, 512]),
                        ("z0", [P, T]), ("zq0", [P, T]), ("v1", [T, D])]:
            d_dbg[nm] = nc.dram_tensor(
                f"dbg_{nm}", shp, f32, kind="ExternalOutput").ap()

    with tile.TileContext(nc) as tc, ExitStack() as ctx:
        wpool = ctx.enter_context(tc.tile_pool(name="weights", bufs=1))
        vpool = ctx.enter_context(tc.tile_pool(name="vpool", bufs=1))
        # xT/xr: 8 tiles per head, both heads live across the whole layer
        xpool = ctx.enter_context(tc.tile_pool(name="xpool", bufs=16))
        xrpool = ctx.enter_context(tc.tile_pool(name="xrpool", bufs=16))
        xspool = ctx.enter_context(tc.tile_pool(name="xspool", bufs=8))
        wvpool = ctx.enter_context(tc.tile_pool(name="wvpool", bufs=2))
        apool = ctx.enter_context(tc.tile_pool(name="apool", bufs=4))
        yrpool = ctx.enter_context(tc.tile_pool(name="yrpool", bufs=3))
        ypool = ctx.enter_context(tc.tile_pool(name="ypool", bufs=4))
        zpool = ctx.enter_context(tc.tile_pool(name="zpool", bufs=1))
        zrpool = ctx.enter_context(tc.tile_pool(name="zrpool", bufs=4))
        zqpool = ctx.enter_context(tc.tile_pool(name="zqpool", bufs=2))
        lnpool = ctx.enter_context(tc.tile_pool(name="lnpool", bufs=3))
        stpool = ctx.enter_context(tc.tile_pool(name="stpool", bufs=4))
        # PSUM budget (8 banks): psX 4x1 + psW 1x2 + psZ 2x1 = 8
        psX = ctx.enter_context(tc.tile_pool(name="psX", bufs=4, space="PSUM"))
        psW = ctx.enter_context(tc.tile_pool(name="psW", bufs=1, space="PSUM"))
        psZ = ctx.enter_context(tc.tile_pool(name="psZ", bufs=2, space="PSUM"))
        dpool = ctx.enter_context(tc.tile_pool(name="drampool", bufs=2, space="DRAM"))

        # ---- persistent weights: direct bf16 DMA, no round-copies ----
        oh_sb = []
        for k in range(ND):
            t_ = wpool.tile([P, T], bf16, tag=f"oh{k}", name=f"oh{k}")
            nc.sync.dma_start(t_[:], d_oh[ts(k, P), :])
            oh_sb.append(t_)
        ew_sb = []
        for k in range(ND):
            t_ = wpool.tile([P, D], bf16, tag=f"ew{k}", name=f"ew{k}")
            nc.sync.dma_start(t_[:], d_ew[ts(k, P), :])
            ew_sb.append(t_)
        cos_sb = []
        sin_sb = []
        for i in range(4):
            t_ = wpool.tile([P, T], bf16, tag=f"cos{i}", name=f"cos{i}")
            nc.sync.dma_start(t_[:], d_cos[ts(i, P), :])
            cos_sb.append(t_)
        for i in range(4):
            t_ = wpool.tile([P, T], bf16, tag=f"sin{i}", name=f"sin{i}")
            nc.sync.dma_start(t_[:], d_sin[ts(i, P), :])
            sin_sb.append(t_)
        dx_sb = []
        for i in range(4):
            t_ = wpool.tile([P, Dh], bf16, tag=f"dx{i}", name=f"dx{i}")
            nc.sync.dma_start(t_[:], d_dx[ts(i, P), :])
            dx_sb.append(t_)
        dy_sb = []
        for i in range(4):
            t_ = wpool.tile([P, Dh], bf16, tag=f"dy{i}", name=f"dy{i}")
            nc.sync.dma_start(t_[:], d_dy[ts(i, P), :])
            dy_sb.append(t_)
        eh_sb = []  # eh_sb[j][k]: E rows for local head j, e-block k
        for j in range(2):
            row = []
            for k in range(NDh):
                t_ = wpool.tile([P, D], bf16, tag=f"eh{j}_{k}", name=f"eh{j}_{k}")
                nc.sync.dma_start(t_[:], d_eh[ts(8 * j + k, P), :])
                row.append(t_)
            eh_sb.append(row)
        ro_sb = []
        for k in range(ND):
            t_ = wpool.tile([P, V], bf16, tag=f"ro{k}", name=f"ro{k}")
            nc.sync.dma_start(t_[:], d_ro[ts(k, P), :])
            ro_sb.append(t_)

        ident = wpool.tile([P, P], f32, tag="ident", name="ident")
        make_identity(nc, ident)
        identb = wpool.tile([P, P], bf16, tag="identb", name="identb")
        nc.scalar.copy(identb[:], ident[:])
        epsc = wpool.tile([P, 1], f32, tag="epsc", name="epsc")
        nc.gpsimd.memset(epsc[:], EPS)
        warmsink = wpool.tile([P, 1], f32, tag="warmsink", name="warmsink")

        def warm(n_mms, label, width=512):
            # HAM re-throttles the PE to 1.2 GHz after ~3.4us idle; feed it
            # dependency-free matmuls during known stall windows.
            wps = psX.tile([P, width], f32, tag="psX", name=f"warm_{label}")
            for i in range(n_mms):
                nc.tensor.matmul(wps[:], dx_sb[0][:, 0:P], dx_sb[1][:, 0:width],
                                 start=(i == 0), stop=(i == n_mms - 1))
            nc.scalar.copy(warmsink[:], wps[:, 0:1])

        def warm_on(dep_ap, label, n_mms=2, width=256):
            # warm matmuls that WAIT on dep_ap: naturally spaced through a
            # stall window by the producer chain of dep_ap.
            wps = psX.tile([P, width], f32, tag="psX", name=f"warmd_{label}")
            for i in range(n_mms):
                nc.tensor.matmul(wps[:], dx_sb[0][:, 0:P], dep_ap,
                                 start=(i == 0), stop=(i == n_mms - 1))
            nc.scalar.copy(warmsink[:], wps[:, 0:1])

        # ---- persistent activations ----
        v_sb = [vpool.tile([P, D], f32, tag=f"v{m}", name=f"v{m}")
                for m in range(NT)]
        v_bf = [vpool.tile([P, D], bf16, tag=f"vb{m}", name=f"vb{m}")
                for m in range(NT)]
        vT_sb = [vpool.tile([P, T], bf16, tag=f"vT{k}", name=f"vT{k}")
                 for k in range(ND)]

        def ln_stats(src_ap):
            st6 = stpool.tile([P, 6], f32, tag="st6", name="st6")
            nc.vector.bn_stats(st6[:], src_ap)
            mv = stpool.tile([P, 2], f32, tag="mv", name="mv")
            nc.vector.bn_aggr(mv[:], st6[:])
            sd = stpool.tile([P, 1], f32, tag="sd", name="sd")
            nc.scalar.activation(sd[:], mv[:, 1:2], AF.Sqrt, bias=epsc[:], scale=1.0)
            rstd = stpool.tile([P, 1], f32, tag="rstd", name="rstd")
            nc.vector.reciprocal(rstd[:], sd[:])
            nmr = stpool.tile([P, 1], f32, tag="nmr", name="nmr")
            nc.vector.scalar_tensor_tensor(
                nmr[:], mv[:, 0:1], -1.0, rstd[:], op0=ALU.mult, op1=ALU.mult)
            return rstd, nmr

        def layer_norm_dual(src_ap, m, zero_mean=False):
            # v_sb[m] (f32, on ACT) and v_bf[m] (bf16, on DVE) from one stats
            # pass. Both applies are off the serial LN chain (consumers are
            # next layer's matmuls). zero_mean: src is a sum of two LN
            # outputs, so its mean is 0 and the applies are scale-only.
            rstd, nmr = ln_stats(src_ap)
            if zero_mean:
                nc.scalar.activation(v_sb[m][:], src_ap, AF.Identity,
                                     bias=0.0, scale=rstd[:])
                nc.vector.tensor_scalar(v_bf[m][:], src_ap, rstd[:], None,
                                        op0=ALU.mult)
            else:
                nc.scalar.activation(v_sb[m][:], src_ap, AF.Identity,
                                     bias=nmr[:], scale=rstd[:])
                nc.vector.tensor_scalar(v_bf[m][:], src_ap, rstd[:], nmr[:],
                                        op0=ALU.mult, op1=ALU.add)

        def dump(dst_ap, src_ap, name="dbg"):
            stg = lnpool.tile([P, src_ap.free_size()], f32, tag="dbgstg",
                              name=f"stg_{name}")
            nc.scalar.copy(stg[:], src_ap)
            nc.sync.dma_start(dst_ap, stg[:])

        def transpose_v_half(n):
            # vT[:, n-half] <- v_bf[4n..4n+3]; per kd one [128,512] bank
            for kd in range(ND):
                tps = psX.tile([P, TH], bf16, tag="psX", name=f"vtp{kd}")
                for mloc in range(4):
                    m = 4 * n + mloc
                    nc.tensor.transpose(
                        tps[:, ts(mloc, P)], v_bf[m][:, ts(kd, P)], identb[:])
                nc.vector.tensor_copy(
                    vT_sb[kd][:, ts(n, TH)].bitcast(u32), tps[:].bitcast(u32))

        # xT[j][mm] / xr[j][mm] for the CURRENT layer (rotated via pools)
        xT = [[None] * NDh, [None] * NDh]
        xr = [[None] * NDh, [None] * NDh]

        def emit_A_half(j, n, layer):
            # x[:, n-half] = relu(Dx^T @ vT[:, n-half]) for head j
            for mp in range(4):
                for mm in (mp, mp + 4):
                    xps = psX.tile([P, TH], f32, tag="psX", name="xps")
                    for k in range(ND):
                        nc.tensor.matmul(
                            xps[:], dx_sb[2 * j + k][:, ts(mm, P)],
                            vT_sb[k][:, ts(n, TH)],
                            start=(k == 0), stop=(k == ND - 1))
                    if n == 0:
                        xT[j][mm] = xpool.tile([P, T], bf16, tag="xT",
                                               name=f"xT{j}_{mm}_{layer}")
                    nc.scalar.activation(xT[j][mm][:, ts(n, TH)], xps[:], AF.Relu)

        def emit_rope_half(j, mp, n, layer):
            # RoPE on the (mp, mp+4) tile pair, token half n, all bf16 DVE:
            #   xr_lo = lo*cos - hi*sin ;  xr_hi = hi*cos + lo*sin
            sl = bass.ts(n, TH)
            cm, sm = cos_sb[mp][:, sl], sin_sb[mp][:, sl]
            lo, hi = xT[j][mp], xT[j][mp + 4]
            if n == 0:
                xr[j][mp] = xrpool.tile([P, T], bf16, tag="xr",
                                        name=f"xr{j}_{mp}_{layer}")
                xr[j][mp + 4] = xrpool.tile([P, T], bf16, tag="xr",
                                            name=f"xr{j}_{mp + 4}_{layer}")
            xrl, xrh = xr[j][mp], xr[j][mp + 4]
            t1 = stpool.tile([P, TH], bf16, tag="ropetmp", name="rt1")
            nc.vector.tensor_mul(t1[:], hi[:, sl], sm)
            nc.vector.tensor_mul(xrl[:, sl], lo[:, sl], cm)
            nc.vector.tensor_sub(xrl[:, sl], xrl[:, sl], t1[:])
            t2 = stpool.tile([P, TH], bf16, tag="ropetmp", name="rt2")
            nc.vector.tensor_mul(t2[:], lo[:, sl], sm)
            nc.vector.tensor_mul(xrh[:, sl], hi[:, sl], cm)
            nc.vector.tensor_add(xrh[:, sl], xrh[:, sl], t2[:])

        def emit_ropes_head(j, layer):
            for n in range(2):
                for mp in range(4):
                    emit_rope_half(j, mp, n, layer)

        # ---- embedding: v0 = LN(onehot @ emb_w) ----
        for m in range(NT):
            eps_t = psX.tile([P, D], f32, tag="psX", name="embps")
            for k in range(ND):
                nc.tensor.matmul(eps_t[:], oh_sb[k][:, ts(m, P)], ew_sb[k][:],
                                 start=(k == 0), stop=(k == ND - 1))
            emb_t = lnpool.tile([P, D], f32, tag="w", name="embt")
            nc.scalar.copy(emb_t[:], eps_t[:])
            layer_norm_dual(emb_t[:], m)
            if DEBUG:
                dump(d_dbg["v0"][ts(m, P), :], v_sb[m][:], name=f"v0_{m}")
        transpose_v_half(0)
        transpose_v_half(1)
        for nn_ in range(2):
            emit_A_half(0, nn_, 0)
            emit_A_half(1, nn_, 0)
        emit_ropes_head(0, 0)

        rg = [[0, 1], [2, 3], [4, 5], [6, 7]]

        # Prime the collective rings during the lead-in: the first real
        # AllReduce otherwise pays a one-time ~13us setup stall.
        pin = dpool.tile([P, P], bf16, tag="prime_i", name="prime_i")
        pout = dpool.tile([P, P], bf16, tag="prime_o", name="prime_o")
        nc.sync.dma_start(pin[:], identb[:])
        nc.gpsimd.collective_compute(
            "AllReduce", mybir.AluOpType.add,
            ins=[pin.opt()], outs=[pout.opt()], replica_groups=rg)

        for layer in range(L):
            z_sb = [zpool.tile([P, T], bf16, tag=f"z{i}", name=f"z{i}_{layer}")
                    for i in range(2)]
            zin = [dpool.tile([2 * P, TH], bf16, tag=f"zin{n}",
                              name=f"zin{n}_{layer}") for n in range(2)]
            zout = [dpool.tile([2 * P, TH], bf16, tag=f"zout{n}",
                               name=f"zout{n}_{layer}") for n in range(2)]
            if DEBUG and layer == 0:
                dump(d_dbg["xr00"][:], xr[0][0][:], name="xr00")

            aT = [[None, None], [None, None]]
            for j in range(2):
                if j == 1:
                    # head1 RoPE just-in-time: its DVE ops queue behind
                    # head0's xrs evictions, ready when BC(j1) needs them
                    emit_ropes_head(1, layer)
                # ---- B/C: xr_std = tp(xr) interleaved k-outer with
                #           Wv = xr^T @ v  (psW pass1 e-blocks 0-3, pass2 4-7)
                xrs = [None] * NT
                wv_sb = [None, None]
                for half in range(2):
                    wvp = psW.tile([P, T], f32, tag="psW", name=f"wv{half}")
                    wvt = wvpool.tile([P, T], bf16, tag="wv", name=f"wv{half}")
                    # eb-outer: each e-block's 8-mm accumulation chain is
                    # contiguous. Interleaving chains that share a PSUM bank
                    # is WRONG: start=True clears has_written for the WHOLE
                    # bank, so a sibling chain's k=0 contribution gets
                    # overwritten at its k=1. Transposes (non-accumulating,
                    # other banks) interleave freely with the first chain.
                    # The tp's for Dh-half `half` are emitted inside pass
                    # `half`, so pass 0 starts as soon as the lo xr tiles
                    # are RoPE'd. Bank 0 (eb 0-1) is evicted while eb 2-3
                    # still accumulate in bank 1 (different banks: legal).
                    for eb in range(4):
                        e = 4 * half + eb
                        for k in range(NT):
                            if eb == 0:
                                tpp = psX.tile([P, TH], bf16, tag="psX",
                                               name=f"tpp{k}")
                                for kk in range(4):
                                    nc.tensor.transpose(
                                        tpp[:, ts(kk, P)],
                                        xr[j][4 * half + kk][:, ts(k, P)],
                                        identb[:])
                                if half == 0:
                                    xrs[k] = xspool.tile(
                                        [P, Dh], bf16, tag="xrs", name=f"xrs{k}")
                                nc.vector.tensor_copy(
                                    xrs[k][:, ts(half, TH)].bitcast(u32),
                                    tpp[:].bitcast(u32))
                                if DEBUG and layer == 0 and j == 0 and k == 0 \
                                        and half == 1:
                                    dump(d_dbg["xrs0"][:], xrs[k][:],
                                         name="xrs0")
                            nc.tensor.matmul(
                                wvp[:, ts(eb, D)],
                                xrs[k][:, ts(e, P)],
                                v_bf[k][:],
                                start=(k == 0), stop=(k == NT - 1))
                        if eb == 1:
                            nc.scalar.copy(wvt[:, 0:TH], wvp[:, 0:TH])
                    nc.scalar.copy(wvt[:, TH:T], wvp[:, TH:T])
                    wv_sb[half] = wvt
                    if DEBUG and layer == 0 and j == 0 and half == 0:
                        dump(d_dbg["wv0"][:], wvt[:], name="wv0")

                # ---- D: aT = (xr @ Wv)^T = Wv^T-as-lhsT @ xr, per n-half ----
                for m in range(ND):
                    aT[j][m] = apool.tile([P, T], bf16, tag="aT",
                                          name=f"aT{j}_{m}_{layer}")
                for n in range(2):
                    for m in range(ND):
                        atp = psX.tile([P, TH], f32, tag="psX", name=f"atp{m}")
                        for k in range(NDh):
                            nc.tensor.matmul(
                                atp[:],
                                wv_sb[k // 4][:, (k % 4) * D + m * P:
                                              (k % 4) * D + (m + 1) * P],
                                xr[j][k][:, ts(n, TH)],
                                start=(k == 0), stop=(k == NDh - 1))
                        nc.scalar.copy(aT[j][m][:, ts(n, TH)], atp[:])
                if DEBUG and layer == 0 and j == 0:
                    dump(d_dbg["aT00"][:], aT[j][0][:], name="aT00")

            # ---- E: y = relu(Dy^T @ aT) * x ; z += E_h^T @ y. n-outer over
            #      both heads so z[:, n0] completes early and its AllReduce
            #      overlaps the n1 compute. ----
            for n in range(2):
                for j in range(2):
                    z_ps = [psZ.tile([P, TH], f32, tag="psZ", name=f"zps{i}")
                            for i in range(2)]
                    y_half = [None] * NDh

                    def emit_zn(k, j=j, z_ps=z_ps, y_half=y_half):
                        for m in range(ND):
                            nc.tensor.matmul(
                                z_ps[m][:],
                                eh_sb[j][k][:, ts(m, P)],
                                y_half[k][:],
                                start=(k == 0), stop=(k == NDh - 1))

                    for k in range(NDh):
                        yps = psX.tile([P, TH], f32, tag="psX", name="yps")
                        for kk in range(ND):
                            nc.tensor.matmul(
                                yps[:],
                                dy_sb[2 * j + kk][:, ts(k, P)],
                                aT[j][kk][:, ts(n, TH)],
                                start=(kk == 0), stop=(kk == ND - 1))
                        yr = yrpool.tile([P, TH], bf16, tag="yr", name=f"yr{k}")
                        nc.scalar.activation(yr[:], yps[:], AF.Relu)
                        yh = ypool.tile([P, TH], bf16, tag="y", name=f"y{k}")
                        nc.vector.tensor_mul(yh[:], yr[:], xT[j][k][:, ts(n, TH)])
                        y_half[k] = yh
                        if DEBUG and layer == 0 and j == 0 and n == 0 and k == 0:
                            dump(d_dbg["y00"][:], yh[:], name="y00")
                        if k > 0:
                            emit_zn(k - 1)
                    emit_zn(NDh - 1)
                    if j == 0:
                        for i in range(2):
                            nc.scalar.copy(z_sb[i][:, ts(n, TH)], z_ps[i][:])
                    else:
                        # the local j0+j1 sum and its DMA gate the AllReduce
                        # launch: keep them ahead of the fusion backlog
                        with tc.high_priority():
                            for i in range(2):
                                nc.vector.scalar_tensor_tensor(
                                    z_sb[i][:, ts(n, TH)], z_ps[i][:], 0.0,
                                    z_sb[i][:, ts(n, TH)],
                                    op0=ALU.add, op1=ALU.add)
                                nc.sync.dma_start(zin[n][ts(i, P), :],
                                                  z_sb[i][:, ts(n, TH)])
                # half-T AllReduce of zT over the core pair, fired as soon
                # as this half's z is complete
                nc.gpsimd.collective_compute(
                    "AllReduce", mybir.AluOpType.add,
                    ins=[zin[n].opt()], outs=[zout[n].opt()],
                    replica_groups=rg)

            if DEBUG and layer == 0:
                dump(d_dbg["z0"][:], z_sb[0][:], name="z0")

            warm(6, f"ar{layer}")

            # ---- boundary: zq transpose + LN chain per token half, with
            #      half n1's LN running on DVE while the PE does half n0's
            #      vT rebuild + next-layer x-phase. PE order: zq0-tp,
            #      (warm-paced LN m0-1), zq1-tp, (LN m2-3), vT0, A'(n0),
            #      vT1, A'(n1). DVE order: zq0-ev, LN m0-3, zq1-ev, vT0-ev,
            #      LN m4-7, vT1-ev, ropes. ----
            last = (layer == L - 1)
            zr = [[zrpool.tile([P, TH], bf16, tag=f"zr{i}",
                               name=f"zr{i}_{n}_{layer}") for i in range(2)]
                  for n in range(2)]
            for n in range(2):
                for i in range(2):
                    nc.sync.dma_start(zr[n][i][:], zout[n][ts(i, P), :])
            zqt = [None, None]
            zS = {}

            def zq_transpose(n):
                zqp = psX.tile([P, T], bf16, tag="psX", name=f"zqp{n}")
                for mloc in range(4):
                    for kd in range(ND):
                        nc.tensor.transpose(
                            zqp[:, mloc * D + kd * P: mloc * D + (kd + 1) * P],
                            zr[n][kd][:, ts(mloc, P)], identb[:])
                zqt[n] = zqpool.tile([P, T], bf16, tag="zq",
                                     name=f"zq{n}_{layer}")
                # even m-blocks (DVE LN chains) evicted on DVE; odd m-blocks
                # (ACT LN chains) on ACT with the row-sum accumulated for
                # their mean as a side effect
                for mloc in (0, 2):
                    nc.vector.tensor_copy(
                        zqt[n][:, ts(mloc, D)].bitcast(u32),
                        zqp[:, ts(mloc, D)].bitcast(u32))
                for mloc in (1, 3):
                    m = 4 * n + mloc
                    S = stpool.tile([P, 1], f32, tag="S", name=f"S{m}")
                    nc.scalar.activation(
                        zqt[n][:, ts(mloc, D)], zqp[:, ts(mloc, D)],
                        AF.Identity, bias=0.0, scale=1.0, accum_out=S[:])
                    zS[m] = S

            def ln_m(m):
                # DVE-resident LN chain (bn_stats path)
                zb = zqt[m // 4][:, ts(m % 4, D)]
                rstd, nmr = ln_stats(zb)
                u = lnpool.tile([P, D], f32, tag="u", name=f"u{m}")
                nc.vector.tensor_scalar(u[:], zb, rstd[:], nmr[:],
                                        op0=ALU.mult, op1=ALU.add)
                w = lnpool.tile([P, D], f32, tag="w", name=f"w{m}")
                nc.vector.tensor_add(w[:], v_sb[m][:], u[:])
                # w = LN(z) + v where v is itself an LN output, so
                # mean(w) = 0 and the second LN is scale-only
                layer_norm_dual(w[:], m, zero_mean=True)
                warm_on(v_bf[m][:], f"ln{layer}_{m}")
                if DEBUG and layer == 0:
                    dump(d_dbg["v1"][ts(m, P), :], v_sb[m][:], name=f"v1_{m}")

            def ln_m_act(m):
                # ACT-resident LN chain, running in parallel with the DVE
                # chains: stats via accum_out row-sums + Square, with mean^2
                # folded into the Sqrt bias. sqrt(SSQ/D + (eps - mean^2))
                # == sqrt(var + eps).
                n, mloc = m // 4, m % 4
                zb = zqt[n][:, ts(mloc, D)]
                S = zS[m]
                sq = lnpool.tile([P, D], f32, tag="sqscr", name=f"sq{m}")
                ssq = stpool.tile([P, 1], f32, tag="SSQ", name=f"SSQ{m}")
                nc.scalar.activation(sq[:], zb, AF.Square, accum_out=ssq[:])
                m2 = stpool.tile([P, 1], f32, tag="m2", name=f"m2{m}")
                nc.scalar.activation(m2[:], S[:], AF.Square, scale=1.0 / D)
                negb = stpool.tile([P, 1], f32, tag="negb", name=f"negb{m}")
                nc.scalar.activation(negb[:], m2[:], AF.Identity,
                                     bias=epsc[:], scale=-1.0)
                sd = stpool.tile([P, 1], f32, tag="sd", name=f"sd{m}")
                nc.scalar.activation(sd[:], ssq[:], AF.Sqrt,
                                     bias=negb[:], scale=1.0 / D)
                rstd = stpool.tile([P, 1], f32, tag="rstd", name=f"rstd{m}")
                nc.vector.reciprocal(rstd[:], sd[:])
                nmr = stpool.tile([P, 1], f32, tag="nmr", name=f"nmr{m}")
                nc.vector.scalar_tensor_tensor(
                    nmr[:], S[:], -1.0 / D, rstd[:], op0=ALU.mult, op1=ALU.mult)
                u = lnpool.tile([P, D], f32, tag="u", name=f"u{m}")
                nc.scalar.activation(u[:], zb, AF.Identity,
                                     bias=nmr[:], scale=rstd[:])
                w = lnpool.tile([P, D], f32, tag="w", name=f"w{m}")
                nc.vector.tensor_add(w[:], v_sb[m][:], u[:])
                # second LN scale-only (mean(w) = 0), stats on ACT
                sqw = lnpool.tile([P, D], f32, tag="sqscr", name=f"sqw{m}")
                ssqw = stpool.tile([P, 1], f32, tag="SSQ", name=f"SSQw{m}")
                nc.scalar.activation(sqw[:], w[:], AF.Square, accum_out=ssqw[:])
                sdw = stpool.tile([P, 1], f32, tag="sd", name=f"sdw{m}")
                nc.scalar.activation(sdw[:], ssqw[:], AF.Sqrt,
                                     bias=epsc[:], scale=1.0 / D)
                rstdw = stpool.tile([P, 1], f32, tag="rstd", name=f"rstdw{m}")
                nc.vector.reciprocal(rstdw[:], sdw[:])
                nc.scalar.activation(v_sb[m][:], w[:], AF.Identity,
                                     bias=0.0, scale=rstdw[:])
                nc.vector.tensor_scalar(v_bf[m][:], w[:], rstdw[:], None,
                                        op0=ALU.mult)
                warm_on(v_bf[m][:], f"lna{layer}_{m}")
                if DEBUG and layer == 0:
                    dump(d_dbg["v1"][ts(m, P), :], v_sb[m][:], name=f"v1_{m}")

            def emit_next_half(n):
                if last:
                    for mloc in range(4):
                        m = 4 * n + mloc
                        rps = psX.tile([P, V], f32, tag="psX", name="rps")
                        for k in range(ND):
                            nc.tensor.matmul(
                                rps[:], vT_sb[k][:, ts(m, P)], ro_sb[k][:],
                                start=(k == 0), stop=(k == ND - 1))
                        o_sb = lnpool.tile([P, V], f32, tag="o", name=f"o{m}")
                        nc.scalar.copy(o_sb[:], rps[:])
                        nc.sync.dma_start(d_out[ts(m, P), :], o_sb[:])
                else:
                    emit_A_half(0, n, layer + 1)
                    emit_A_half(1, n, layer + 1)

            # The LN chain is the serial critical path of the boundary; give
            # it scheduler priority so the ready-heap doesn't interleave
            # next-layer RoPE/relu work ahead of it on DVE/ACT. Each half's
            # four chains alternate DVE (bn_stats) / ACT (accum-sum stats)
            # so the two engines work the chains in parallel.
            with tc.high_priority():
                zq_transpose(0)
                ln_m(0)
                ln_m_act(1)
                ln_m(2)
                ln_m_act(3)
                if DEBUG and layer == 0:
                    dump(d_dbg["zq0"][:], zqt[0][:], name="zq0")
                zq_transpose(1)
                transpose_v_half(0)
            emit_next_half(0)
            with tc.high_priority():
                ln_m(4)
                ln_m_act(5)
                ln_m(6)
                ln_m_act(7)
                transpose_v_half(1)
            emit_next_half(1)
            if not last:
                emit_ropes_head(0, layer + 1)

    nc.compile()
    return nc


def _get_program():
    if "nc" not in _CACHE:
        _CACHE["nc"] = _build_program()
    return _CACHE["nc"]


def _rope_tables():
    inv = (1.0 / (10000.0 ** (np.arange(0, Dh, 2, dtype=np.float32) / Dh)))
    tt = np.arange(T, dtype=np.float32)
    freqs = np.outer(tt, inv).astype(np.float32)  # [T, Dh/2]
    cosT = np.ascontiguousarray(np.cos(freqs).T)
    sinT = np.ascontiguousarray(np.sin(freqs).T)
    return cosT, sinT


def kernel(**inputs):
    global LAST_RESULT
    import ml_dtypes
    from concourse import bass_utils

    bf = ml_dtypes.bfloat16
    tokens = np.asarray(inputs["tokens"])
    emb_w = np.ascontiguousarray(inputs["emb_w"], dtype=np.float32)
    E = np.ascontiguousarray(inputs["E"], dtype=np.float32)
    Dx = np.ascontiguousarray(inputs["Dx"], dtype=np.float32)
    Dy = np.ascontiguousarray(inputs["Dy"], dtype=np.float32)
    readout = np.ascontiguousarray(inputs["readout"], dtype=np.float32)

    cosT, sinT = _rope_tables()

    in_maps = []
    for c in range(NCORES):
        b, hp = c // 2, c % 2
        oh = np.zeros((V, T), dtype=np.float32)
        oh[np.asarray(tokens[b], dtype=np.int64), np.arange(T)] = 1.0
        in_maps.append({
            "onehotT": oh.astype(bf),
            "emb_w": emb_w.astype(bf),
            "cosT": cosT.astype(bf),
            "sinT": sinT.astype(bf),
            "dx": np.ascontiguousarray(
                Dx[2 * hp:2 * hp + 2].reshape(2 * D, Dh)).astype(bf),
            "dy": np.ascontiguousarray(
                Dy[2 * hp:2 * hp + 2].reshape(2 * D, Dh)).astype(bf),
            "eh": np.ascontiguousarray(
                E[2 * hp * Dh:(2 * hp + 2) * Dh]).astype(bf),
            "readout": readout.astype(bf),
        })

    nc = _get_program()
    res = bass_utils.run_bass_kernel_spmd(
        nc, in_maps, core_ids=list(range(NCORES)),
        trace=bool(int(os.environ.get("KERNEL_TRACE", "0"))))
    LAST_RESULT = res
    out = np.stack([res.results[2 * b]["out"] for b in range(B)], axis=0)
    return out
